# revision 27
# baseline (speedup 1.0000x reference)
"""GAT (3-layer, 3-head) GNN forward on 8 Trainium2 NeuronCores.

Strategy (v2):
- Host partitions the 64 graphs onto 8 cores (8 graphs each); node slots
  padded per graph to a uniform stride so the SPMD program is identical.
- All PE matmuls run in fp16 (1 cycle/row vs 4 for fp32); PSUM stays f32.
- Per layer: phase-1 matmul computes LOCAL node rows [h | es] into a fp16
  DRAM table; the table is AllGathered in S_AG pipelined sub-collectives
  (table ids are sub-block-major so each sub-AllGather's output region is
  contiguous), overlapping the collective with phase-1 compute; BN of the
  previous layer is folded into the weights.
- Aggregation: per 128-dst chunk, dma_gather of h[src] rows; one-hot
  edge->dst matrices are precomputed on host (fp16) and streamed from DRAM
  (both orientations); softmax numerators on ACT with a -8 bias inside exp
  (cancels in the division); weighted one-hots (alpha * onehot) computed
  with wide 4x-mode DVE tensor_scalar ops; scatter-add via fp16 PE matmuls
  accumulating in f32 PSUM; one fused relu*recip finalize op per chunk.
- BN stats: one bn_stats pass over the full layer output at layer end
  (6-tuple -> sum/sumsq algebra), AllReduce, fold into next weights.
- Head: pooling via fixed-stride free-dim reduces, small f32 matmuls, BN4.
"""

import os
import numpy as np

H = 3
NEG_SLOPE = 0.2
BN_EPS = 1e-5
NCORES = 8
EXP_BIAS = -8.0  # subtracted inside exp; cancels in softmax division
S_AG = 2         # sub-AllGathers per layer (pipelined with phase1)

ROW1, ROW23 = 896, 512   # fp16 table row strides (multiples of 128 elems)
CO1, CO23 = 774, 390     # phase-1 matmul widths: h | es(3) | ed(3)


def _ceil_to(x, m):
    return ((x + m - 1) // m) * m


def _prep(inputs):
    """Host-side preprocessing: partitioning, index arrays, one-hot tables,
    combined fp16 weight tables."""
    P = {}
    x = np.asarray(inputs["x"], np.float32)
    ei = np.asarray(inputs["edge_index"])
    batch = np.asarray(inputs["batch"]).astype(np.int64)
    N, F_IN = x.shape
    G = int(batch.max()) + 1
    assert G % NCORES == 0
    GPC = G // NCORES  # graphs per core

    counts = np.bincount(batch, minlength=G)
    gstart = np.concatenate([[0], np.cumsum(counts)[:-1]])
    GSTRIDE = _ceil_to(int(counts.max()), max(16, 128 // GPC))
    PAD_N = GPC * GSTRIDE
    assert PAD_N % 128 == 0
    R = NCORES * PAD_N  # total table rows
    assert R < 32768, f"table rows {R} exceed int16 range"
    CHUNKS = PAD_N // 128
    assert GSTRIDE <= 512, "bn_stats group limit"

    # Balance edge load: graphs sorted by edge count; rank r graph goes to
    # core r%8, slot r//8, so every core's slot s holds a similarly-sized
    # graph and the per-chunk max-over-cores tile count tracks the mean.
    ecnt = np.bincount(batch[np.asarray(ei[1])], minlength=G)
    order = np.argsort(-ecnt, kind="stable")
    gcore = np.empty(G, np.int64)
    gslot = np.empty(G, np.int64)
    gcore[order] = np.arange(G) % NCORES
    gslot[order] = np.arange(G) // NCORES

    n = np.arange(N)
    g = batch
    slot = gcore[g] * PAD_N + gslot[g] * GSTRIDE + (n - gstart[g])
    P.update(N=N, G=G, GPC=GPC, GSTRIDE=GSTRIDE, PAD_N=PAD_N, R=R,
             CHUNKS=CHUNKS, F_IN=F_IN, slot=slot)

    # --- sub-AllGather blocks (tile-aligned); table ids are block-major so
    # each sub-AllGather writes one contiguous region of the full table.
    LNT = CHUNKS
    tsplit = [LNT // S_AG + (1 if i < LNT % S_AG else 0) for i in range(S_AG)]
    rows_s = np.array([t * 128 for t in tsplit], np.int64)
    rowstart = np.concatenate([[0], np.cumsum(rows_s)[:-1]])
    gbase = np.concatenate([[0], np.cumsum(rows_s * NCORES)[:-1]])
    P["ag_tiles"] = tsplit
    P["ag_rowstart"] = rowstart
    P["ag_gbase"] = gbase

    def table_id(sl):
        c, r = sl // PAD_N, sl % PAD_N
        s = np.searchsorted(rowstart, r, side="right") - 1
        return gbase[s] + c * rows_s[s] + (r - rowstart[s])

    # --- edges with self loops, partitioned by dst core, sorted by dst slot
    src = np.concatenate([ei[0], n]).astype(np.int64)
    dst = np.concatenate([ei[1], n]).astype(np.int64)
    sslot = slot[src]
    dslot = slot[dst]
    dcore = dslot // PAD_N

    per_core = []
    for c in range(NCORES):
        m = dcore == c
        ss, dd = table_id(sslot[m]), dslot[m] - c * PAD_N
        order = np.argsort(dd, kind="stable")
        per_core.append((ss[order], (dd[order]) % 128, dd[order] // 128))

    tiles_per_chunk = np.zeros(CHUNKS, np.int64)
    for c in range(NCORES):
        _, _, ck = per_core[c]
        cnt = np.bincount(ck, minlength=CHUNKS)
        tiles_per_chunk = np.maximum(tiles_per_chunk, (cnt + 127) // 128)
    TILES = int(tiles_per_chunk.sum())
    tile_base = np.concatenate([[0], np.cumsum(tiles_per_chunk)[:-1]])
    P["tiles_per_chunk"] = tiles_per_chunk
    P["tile_base"] = tile_base
    P["TILES"] = TILES

    # padded per-core arrays: src table ids (dummy 0), dst offsets (-1)
    src_pad = np.zeros((NCORES, TILES * 128), np.int16)
    dst_pad = np.full((NCORES, TILES * 128), -1, np.int64)
    for c in range(NCORES):
        ss, doff, ck = per_core[c]
        cnt = np.bincount(ck, minlength=CHUNKS)
        off = np.concatenate([[0], np.cumsum(cnt)[:-1]])
        for k in range(CHUNKS):
            nk = int(cnt[k])
            if nk == 0:
                continue
            b = int(tile_base[k]) * 128
            src_pad[c, b:b + nk] = ss[off[k]:off[k] + nk]
            dst_pad[c, b:b + nk] = doff[off[k]:off[k] + nk]

    # wrap src ids for dma_gather: idx i of a chunk-gather at [i%16, i//16],
    # columns grouped per chunk; replicated to 128 partitions.
    IDXC = TILES * 8
    src16 = np.zeros((NCORES, 16, IDXC), np.int16)
    for c in range(NCORES):
        for k in range(CHUNKS):
            t0, nt = int(tile_base[k]), int(tiles_per_chunk[k])
            if nt == 0:
                continue
            seg = src_pad[c, t0 * 128:(t0 + nt) * 128]
            src16[c, :, t0 * 8:(t0 + nt) * 8] = seg.reshape(-1, 16).T
    P["src16"] = np.ascontiguousarray(np.tile(src16, (1, 8, 1)))
    P["IDXC"] = IDXC

    # one-hot edge->dst matrices, both orientations, fp16:
    #   o01[e, t*128+d] = 1 iff edge (t,e) targets chunk-local dst d
    #   o1t[d, t*128+e] = same, transposed per tile
    o01 = np.zeros((NCORES, 128, TILES * 128), np.float16)
    o1t = np.zeros((NCORES, 128, TILES * 128), np.float16)
    idx = np.arange(TILES * 128)
    ee, tt = idx % 128, idx // 128
    for c in range(NCORES):
        d = dst_pad[c]
        v = d >= 0
        o01[c, ee[v], tt[v] * 128 + d[v]] = 1.0
        o1t[c, d[v], tt[v] * 128 + ee[v]] = 1.0
    P["o01"] = o01
    P["o1t"] = o1t

    # --- x^T with ones row, slotted, per-core local slice, fp16
    xT = np.zeros((F_IN + 1, R), np.float16)
    xT[:F_IN, slot] = x.T.astype(np.float16)
    xT[F_IN, slot] = 1.0
    P["x1T"] = np.ascontiguousarray(
        xT.reshape(F_IN + 1, NCORES, PAD_N).transpose(1, 0, 2))

    # --- combined weight tables [K(+1), C*H+6] fp16
    def comb(W, a_s, a_d, b, C, with_bias_row):
        K = W.shape[0]
        rows = K + (1 if with_bias_row else 0)
        Wc = np.zeros((rows, H * C + 6), np.float32)
        Wc[:K, :H * C] = W
        for j in range(H):
            Wc[:K, H * C + j] = W[:, j * C:(j + 1) * C] @ a_s[j]
            Wc[:K, H * C + 3 + j] = W[:, j * C:(j + 1) * C] @ a_d[j]
        if with_bias_row:
            Wc[K, :H * C] = b
        return Wc.astype(np.float16)

    # L1: bias row is matched against the ones row inside x1T (row F_IN),
    # so fold it into the K rows directly at index F_IN.
    W1c = np.zeros((F_IN + 1, H * 256 + 6), np.float32)
    W1c[:F_IN] = comb(np.asarray(inputs["W1"], np.float32),
                      np.asarray(inputs["a1_src"], np.float32),
                      np.asarray(inputs["a1_dst"], np.float32),
                      None, 256, False).astype(np.float32)
    W1c[F_IN, :H * 256] = np.asarray(inputs["b1"], np.float32)
    P["W1c"] = W1c.astype(np.float16)
    P["W2c"] = comb(np.asarray(inputs["W2"], np.float32),
                    np.asarray(inputs["a2_src"], np.float32),
                    np.asarray(inputs["a2_dst"], np.float32),
                    np.asarray(inputs["b2"], np.float32), 128, True)
    P["W3c"] = comb(np.asarray(inputs["W3"], np.float32),
                    np.asarray(inputs["a3_src"], np.float32),
                    np.asarray(inputs["a3_dst"], np.float32),
                    np.asarray(inputs["b3"], np.float32), 128, True)

    def bnpack(gv, bv, nblk):
        t = np.zeros((128, 2 * nblk), np.float32)
        gv = np.asarray(gv, np.float32)
        bv = np.asarray(bv, np.float32)
        for b in range(nblk):
            sg = gv[b * 128:(b + 1) * 128]
            t[:len(sg), b] = sg
            sb = bv[b * 128:(b + 1) * 128]
            t[:len(sb), nblk + b] = sb
        return t

    P["bn1"] = bnpack(inputs["g1"], inputs["be1"], 6)
    P["bn2"] = bnpack(inputs["g2"], inputs["be2"], 3)
    P["bn3"] = bnpack(inputs["g3"], inputs["be3"], 3)
    bn4 = np.zeros((64, 2), np.float32)
    bn4[:, 0] = np.asarray(inputs["g4"], np.float32)
    bn4[:, 1] = np.asarray(inputs["be4"], np.float32)
    P["bn4"] = bn4

    P["fc1w"] = np.asarray(inputs["fc1_w"], np.float32)
    P["fc2w"] = np.asarray(inputs["fc2_w"], np.float32)
    P["fc3w"] = np.asarray(inputs["fc3_w"], np.float32)
    fcb = np.zeros((64, 3), np.float32)
    fcb[:, 0] = np.asarray(inputs["fc1_b"], np.float32)
    fcb[:, 1] = np.asarray(inputs["fc2_b"], np.float32)
    fcb[0, 2] = float(np.asarray(inputs["fc3_b"], np.float32).reshape(-1)[0])
    P["fcb"] = fcb

    g_at = np.empty((NCORES, GPC), np.int64)
    g_at[gcore, gslot] = np.arange(G)
    cntb = np.zeros((NCORES, 128, GPC), np.float32)
    for c in range(NCORES):
        cntb[c, :, :] = counts[g_at[c]][None, :]
    P["cntb"] = cntb
    P["gpos"] = gcore * GPC + gslot  # graph g's position in the raw output
    P["ones1"] = np.ones((1, 128), np.float16)
    e3 = np.zeros((3, 384), np.float16)
    for j in range(3):
        e3[j, j * 128:(j + 1) * 128] = 1.0
    P["e3"] = e3
    P["i128"] = np.eye(128, dtype=np.float16)
    return P


def _build(nc, P, mybir, tile, bass, library_config):
    STAGE = int(os.environ.get("GAT_STAGE", "99"))
    OW_BCAST = int(os.environ.get("GAT_OW_BCAST", "0"))
    dt = mybir.dt
    f32 = dt.float32
    f16 = dt.float16
    AT = mybir.ActivationFunctionType
    OP = mybir.AluOpType
    AX = mybir.AxisListType
    R, PAD_N, CHUNKS, TILES = P["R"], P["PAD_N"], P["CHUNKS"], P["TILES"]
    IDXC, F_IN = P["IDXC"], P["F_IN"]
    GST, GPC = P["GSTRIDE"], P["GPC"]
    LNT = PAD_N // 128
    tiles_per_chunk = P["tiles_per_chunk"]
    tile_base = P["tile_base"]
    N_REAL, G = P["N"], P["G"]
    TMAX = int(tiles_per_chunk.max())
    GROUPS = [list(range(NCORES))]
    ag_tiles = P["ag_tiles"]
    ag_rowstart = P["ag_rowstart"]
    ag_gbase = P["ag_gbase"]

    def block_of_tile(nt):
        r = nt * 128
        s = 0
        while s + 1 < S_AG and r >= ag_rowstart[s + 1]:
            s += 1
        return s

    # ---------------- DRAM tensors ----------------
    def ein(name, shape, dtype=f32):
        return nc.dram_tensor(name, list(shape), dtype, kind="ExternalInput").ap()

    x1T_d = ein("x1T", [F_IN + 1, PAD_N], f16)
    W1c_d = ein("W1c", P["W1c"].shape, f16)
    W2c_d = ein("W2c", P["W2c"].shape, f16)
    W3c_d = ein("W3c", P["W3c"].shape, f16)
    src16_d = ein("src16", [128, IDXC], dt.int16)
    o01_d = ein("o01", [128, TILES * 128], f16)
    o1t_d = ein("o1t", [128, TILES * 128], f16)
    ones1_d = ein("ones1", [1, 128], f16)
    e3_d = ein("e3", [3, 384], f16)
    i128_d = ein("i128", [128, 128], f16)
    bn1_d = ein("bn1", [128, 12])
    bn2_d = ein("bn2", [128, 6])
    bn3_d = ein("bn3", [128, 6])
    bn4_d = ein("bn4", [64, 2])
    fc1w_d = ein("fc1w", [384, 64])
    fc2w_d = ein("fc2w", [64, 64])
    fc3w_d = ein("fc3w", [64, 1])
    fcb_d = ein("fcb", [64, 3])
    cntb_d = ein("cntb", [128, GPC])
    y_d = nc.dram_tensor("y", [1, GPC], f32, kind="ExternalOutput").ap()

    # per-sub-block local tables (fine-grained collective deps) + full tables
    def loc_tbls(lname, row):
        return [nc.dram_tensor(f"{lname}_{s}", [int(ag_tiles[s]) * 128, row],
                               f16).ap() for s in range(S_AG)]

    h1_locs = loc_tbls("h1_loc", ROW1)
    h1_full = nc.dram_tensor("h1_full", [R, ROW1], f16, addr_space="Shared").ap()
    h2_locs = loc_tbls("h2_loc", ROW23)
    h2_full = nc.dram_tensor("h2_full", [R, ROW23], f16, addr_space="Shared").ap()
    h3_locs = loc_tbls("h3_loc", ROW23)
    h3_full = nc.dram_tensor("h3_full", [R, ROW23], f16, addr_space="Shared").ap()
    st_loc = [nc.dram_tensor(f"st{l}_loc", [128, 12], f32).ap() for l in range(3)]
    st_full = [nc.dram_tensor(f"st{l}_full", [128, 12], f32,
                              addr_space="Shared").ap() for l in range(3)]
    st4_loc = nc.dram_tensor("st4_loc", [64, 2], f32).ap()
    st4_full = nc.dram_tensor("st4_full", [64, 2], f32, addr_space="Shared").ap()

    with tile.TileContext(nc) as tc:
        nc.gpsimd.load_library(library_config.mlp)

        with tc.tile_pool(name="pers", bufs=1) as pers:
            ones1 = pers.tile([1, 128], f16, tag="ones1")
            src16 = pers.tile([128, IDXC], dt.int16, tag="src16")
            esed = pers.tile([128, LNT, 6], f16, tag="esed")
            sc1 = pers.tile([128, 6], f32, tag="sc1")
            sh1 = pers.tile([128, 6], f32, tag="sh1")
            sc2 = pers.tile([128, 3], f32, tag="sc2")
            sh2 = pers.tile([128, 3], f32, tag="sh2")
            sc3 = pers.tile([128, 3], f32, tag="sc3")
            sh3 = pers.tile([128, 3], f32, tag="sh3")
            nbias = pers.tile([128, 1], f32, tag="nbias")
            e3 = pers.tile([3, 384], f16, tag="e3")
            i128 = pers.tile([128, 128], f16, tag="i128")
            nc.sync.dma_start(out=ones1[:], in_=ones1_d[:])
            nc.sync.dma_start(out=src16[:], in_=src16_d[:])
            nc.sync.dma_start(out=e3[:], in_=e3_d[:])
            nc.sync.dma_start(out=i128[:], in_=i128_d[:])
            nc.vector.memset(nbias[:], EXP_BIAS)

            # =====================================================
            def phase1(xsrc, kdims, Wc_d, CO, ROW, tbls, with_ones,
                       sc_in, sh_in, tagp):
                """h_tile = lhs @ Wc (+ ones*wbot); writes [h|es] fp16 rows to
                the per-block tbls and es|ed to esed. If sc_in/sh_in given,
                folds the previous layer's BN into the weights first.
                xsrc: None (load x1T from DRAM) or a [128, nK, PAD_N] tile."""
                nK = len(kdims)
                n_mm = (CO + 511) // 512
                with tc.tile_pool(name=f"w{tagp}", bufs=1) as wp, \
                     tc.tile_pool(name=f"p1{tagp}", bufs=3) as sp, \
                     tc.tile_pool(name=f"ps{tagp}", bufs=2, space="PSUM") as pp, \
                     tc.tile_pool(name=f"x{tagp}", bufs=1) as xp:
                    Wts = []
                    r0 = 0
                    for ki, kd in enumerate(kdims):
                        wt = wp.tile([kd, CO], f16, tag=f"w{ki}")
                        nc.sync.dma_start(out=wt[:], in_=Wc_d[r0:r0 + kd, :])
                        Wts.append(wt)
                        r0 += kd
                    wbot = None
                    if with_ones:
                        wbot_raw = wp.tile([1, CO], f16, tag="wbotr")
                        nc.sync.dma_start(out=wbot_raw[:], in_=Wc_d[r0:r0 + 1, :])
                        wbot = wbot_raw
                    if sc_in is not None:
                        # bias fix first (uses raw W): wbot' = wbot + sh @ W
                        shh = wp.tile([128, nK], f16, tag="shh")
                        nc.vector.tensor_copy(shh[:], sh_in[:, :nK])
                        pb = pp.tile([1, CO], f32, tag="pb")
                        for ki in range(nK):
                            nc.tensor.matmul(pb[:], shh[:, ki:ki + 1], Wts[ki][:],
                                             start=(ki == 0), stop=(ki == nK - 1))
                        wbot2 = wp.tile([1, CO], f16, tag="wbot2")
                        nc.vector.tensor_tensor(wbot2[:], wbot_raw[:], pb[:],
                                                OP.add)
                        wbot = wbot2
                        # then scale rows in place: W'[k,:] = sc_k * W[k,:]
                        for ki in range(nK):
                            nc.vector.tensor_scalar(
                                Wts[ki][:], Wts[ki][:], sc_in[:kdims[ki],
                                                              ki:ki + 1],
                                None, OP.mult)

                    if xsrc is None:
                        xg = []
                        r0 = 0
                        for ki, kd in enumerate(kdims):
                            t = xp.tile([kd, PAD_N], f16, tag=f"xg{ki}")
                            nc.sync.dma_start(out=t[:],
                                              in_=x1T_d[r0:r0 + kd, :])
                            xg.append(t)
                            r0 += kd

                    for nt in range(LNT):
                        if xsrc is None:
                            lhs = [xg[ki][:, nt * 128:(nt + 1) * 128]
                                   for ki in range(nK)]
                        else:
                            lhs = [xsrc[:, ki, nt * 128:(nt + 1) * 128]
                                   for ki in range(nK)]
                        hp = pp.tile([128, CO], f32, tag="hp")
                        for ki in range(nK):
                            for mi in range(n_mm):
                                c0, c1 = mi * 512, min(CO, mi * 512 + 512)
                                nc.tensor.matmul(
                                    hp[:, c0:c1], lhs[ki], Wts[ki][:, c0:c1],
                                    start=(ki == 0),
                                    stop=(not with_ones and ki == nK - 1))
                        if with_ones:
                            for mi in range(n_mm):
                                c0, c1 = mi * 512, min(CO, mi * 512 + 512)
                                nc.tensor.matmul(
                                    hp[:, c0:c1], ones1[:], wbot[:, c0:c1],
                                    start=False, stop=True)
                        hs = sp.tile([128, ROW], f16, tag="hs")
                        if nt % 2 == 0:
                            nc.vector.tensor_copy(hs[:, :CO - 3], hp[:, :CO - 3])
                        else:
                            nc.scalar.copy(hs[:, :CO - 3], hp[:, :CO - 3])
                        nc.vector.memset(hs[:, CO - 3:ROW], 0.0)
                        nc.vector.tensor_copy(esed[:, nt, :], hp[:, CO - 6:CO])
                        s = block_of_tile(nt)
                        r0 = nt * 128 - int(ag_rowstart[s])
                        nc.sync.dma_start(
                            out=tbls[s][r0:r0 + 128, :], in_=hs[:])

            def allgather_tbl(tbls, full, ROW, tagp):
                for s in range(S_AG):
                    rows = int(ag_tiles[s]) * 128
                    g0 = int(ag_gbase[s])
                    nc.gpsimd.collective_compute(
                        "AllGather", OP.bypass, replica_groups=GROUPS,
                        ins=[tbls[s][:]],
                        outs=[full[g0:g0 + NCORES * rows, :]])

            # =====================================================
            def aggregate(tbl, ROW, C, yT, NB, tagp):
                """Segment-softmax attention + scatter-add for one layer.
                yT: [128, NB, PAD_N] fp16 output tile (feature-major).
                Two-stage software pipeline: stage A (gather + edge scores +
                denominators) of chunk k+1 is emitted before stage B
                (weighting + scatter-add + finalize) of chunk k so the
                in-order engine queues interleave the two chunks."""
                nbh = C // 128
                with tc.tile_pool(name=f"g{tagp}", bufs=4) as gp, \
                     tc.tile_pool(name=f"o{tagp}", bufs=4) as op_, \
                     tc.tile_pool(name=f"a{tagp}", bufs=3) as ap, \
                     tc.tile_pool(name=f"q{tagp}", bufs=2, space="PSUM") as qa, \
                     tc.tile_pool(name=f"e{tagp}", bufs=2, space="PSUM") as qe, \
                     tc.tile_pool(name=f"r{tagp}", bufs=1, space="PSUM") as qs:

                    def stageA1(k):
                        """Gather + one-hot DMAs + dst-score spread (no DVE,
                        no dependency on the gather)."""
                        T = int(tiles_per_chunk[k])
                        if T == 0:
                            nc.vector.memset(yT[:, :, k * 128:(k + 1) * 128],
                                             0.0)
                            return None
                        t0 = int(tile_base[k])
                        o01c = op_.tile([128, TMAX * 128], f16, tag="o01")
                        o1tc = op_.tile([128, TMAX * 128], f16, tag="o1t")
                        nc.sync.dma_start(
                            out=o01c[:, :T * 128],
                            in_=o01_d[:, t0 * 128:(t0 + T) * 128])
                        nc.sync.dma_start(
                            out=o1tc[:, :T * 128],
                            in_=o1t_d[:, t0 * 128:(t0 + T) * 128])
                        hg = gp.tile([128, TMAX, ROW], f16, tag="hg")
                        nidx = T * 128
                        nc.gpsimd.dma_gather(
                            hg[:, :T, :], tbl[:], src16[:, t0 * 8:(t0 + T) * 8],
                            nidx, nidx, ROW, single_packet=False)
                        edv = esed[:, k, 3:6]
                        edb = qe.tile([128, TMAX, 3], f32, tag="scr")
                        for t in range(T):
                            nc.tensor.matmul(edb[:, t, :],
                                             o1tc[:, t * 128:(t + 1) * 128],
                                             edv, start=(t == 0),
                                             stop=(t == T - 1))
                        return dict(T=T, k=k, o01c=o01c, o1tc=o1tc, hg=hg,
                                    edb=edb)

                    def stageA2(S):
                        """Edge softmax numerators + denominators (consumes
                        the gather; runs one pipeline stage later)."""
                        T, hg, edb, o01c = S["T"], S["hg"], S["edb"], S["o01c"]
                        # de-stride the gathered src scores on ACT, add the
                        # dst spread on DVE, then exp(leaky_relu(x)) =
                        # max(exp(x), exp(0.2x)): two fused ACT exps and a
                        # cheap DVE max instead of a DVE leaky-relu
                        esg = ap.tile([128, TMAX, 3], f32, tag="esg")
                        nc.scalar.copy(esg[:, :T, :],
                                       hg[:, :T, H * C:H * C + 3])
                        ex0 = ap.tile([128, TMAX, 3], f32, tag="ex0")
                        nc.vector.tensor_tensor(
                            ex0[:, :T, :], esg[:, :T, :], edb[:, :T, :],
                            OP.add)
                        e1 = ap.tile([128, TMAX, 3], f32, tag="e1")
                        nc.scalar.activation(e1[:, :T, :], ex0[:, :T, :],
                                             AT.Exp, bias=nbias[:])
                        exc = ap.tile([128, TMAX, 3], f32, tag="exc")
                        nc.scalar.activation(exc[:, :T, :], ex0[:, :T, :],
                                             AT.Exp, bias=nbias[:],
                                             scale=NEG_SLOPE)
                        nc.vector.tensor_tensor(exc[:, :T, :], e1[:, :T, :],
                                                exc[:, :T, :], OP.max)
                        # saturating fp16 cast: pad-edge slots read row 0's
                        # src score, whose exp can overflow fp16; inf would
                        # turn the zero one-hot column into NaN (0 * inf)
                        exch = ap.tile([128, TMAX, 3], f16, tag="exch")
                        nc.vector.tensor_scalar_min(exch[:, :T, :],
                                                    exc[:, :T, :], 60000.0)
                        sp_ = qs.tile([3, 128], f32, tag="sp")
                        for t in range(T):
                            nc.tensor.matmul(sp_[:], exch[:, t, :],
                                             o01c[:, t * 128:(t + 1) * 128],
                                             start=(t == 0), stop=(t == T - 1))
                        sr = ap.tile([3, 128], f32, tag="sr")
                        nc.vector.reciprocal(sr[:], sp_[:])
                        # pad dsts have s=0 -> recip inf, and 0*inf = NaN;
                        # clamp (real recips are <= ~3.3e4)
                        nc.vector.tensor_scalar_min(sr[:], sr[:], 60000.0)
                        srh = ap.tile([3, 128], f16, tag="srh")
                        nc.vector.tensor_copy(srh[:], sr[:])
                        S["exch"] = exch
                        S["srh"] = srh

                    def stageB(S):
                        T, o01c, hg, k = S["T"], S["o01c"], S["hg"], S["k"]
                        # alpha-weighted one-hots: one wide broadcast op per
                        # head (in1 free-broadcast of the per-edge alpha)
                        ow = ap.tile([128, TMAX * 3 * 128], f16, tag="ow")
                        o3 = o01c[:, :T * 128].rearrange(
                            "p (t d) -> p t d", t=T)
                        ow4 = ow[:, :T * 384].rearrange(
                            "p (t h d) -> p t h d", h=3, d=128)
                        for h in range(H):
                            nc.vector.tensor_tensor(
                                ow4[:, :, h, :], o3,
                                S["exch"][:, :T, h:h + 1].broadcast_to(
                                    (128, T, 128)), OP.mult)
                        aggp = qa.tile([128, NB, 128], f32, tag="aggp")
                        for t in range(T):
                            for h in range(H):
                                for cb in range(nbh):
                                    fb = h * nbh + cb
                                    # psum groups are per 2KB bank (4 fb
                                    # slices): start/stop only on the bank's
                                    # first/last matmul
                                    st = (t == 0) and (fb % 4 == 0)
                                    sp2 = (t == T - 1) and (
                                        fb % 4 == 3 or fb == NB - 1)
                                    nc.tensor.matmul(
                                        aggp[:, fb, :],
                                        hg[:, t,
                                           h * C + cb * 128:h * C + (cb + 1) * 128],
                                        ow[:, (t * 3 + h) * 128:
                                           (t * 3 + h + 1) * 128],
                                        start=st, stop=sp2)
                        rb = qs.tile([128, 384], f32, tag="rb")
                        for h in range(H):
                            nc.tensor.matmul(rb[:, h * 128:(h + 1) * 128],
                                             e3[:, h * 128:(h + 1) * 128],
                                             S["srh"][:], start=(h == 0),
                                             stop=(h == H - 1))
                        rbs = ap.tile([128, 384], f32, tag="rbs")
                        nc.scalar.copy(rbs[:], rb[:])
                        # fused finalize: yT = relu(aggp) * recip(s);
                        # one 3D op per feature sub-block (verifier caps
                        # tensor_scalar APs at 3 dims)
                        r3 = rbs[:].rearrange("p (h d) -> p h d", h=3)
                        for cb in range(nbh):
                            nc.vector.scalar_tensor_tensor(
                                yT[:, cb:NB:nbh, k * 128:(k + 1) * 128],
                                aggp[:, cb:NB:nbh, :], 0.0, r3,
                                OP.max, OP.mult)

                    states = {}
                    for i in range(CHUNKS + 2):
                        if i < CHUNKS:
                            states[i] = stageA1(i)
                        if 0 <= i - 1 < CHUNKS and states[i - 1] is not None:
                            stageA2(states[i - 1])
                        if 0 <= i - 2 < CHUNKS and states[i - 2] is not None:
                            stageB(states.pop(i - 2))

            # =====================================================
            def layer_stats(yT, NB, bn_d, stl, stf, sc_out, sh_out, tagp):
                """Per-feature sum/sumsq via ACT accum_out -> AllReduce ->
                sc/sh fold coefficients."""
                with tc.tile_pool(name=f"b{tagp}", bufs=1) as bp:
                    scr = bp.tile([128, PAD_N], f16, tag="scr")
                    st2 = bp.tile([128, 12], f32, tag="st2")
                    nc.vector.memset(st2[:], 0.0)
                    for fb in range(NB):
                        nc.scalar.activation(scr[:], yT[:, fb, :], AT.Identity,
                                             accum_out=st2[:, fb:fb + 1])
                        nc.scalar.activation(scr[:], yT[:, fb, :], AT.Square,
                                             accum_out=st2[:, 6 + fb:7 + fb])
                    nc.sync.dma_start(out=stl[:], in_=st2[:])
                    nc.gpsimd.collective_compute(
                        "AllReduce", OP.add, replica_groups=GROUPS,
                        ins=[stl[:]], outs=[stf[:]])
                    stg = bp.tile([128, 12], f32, tag="stg")
                    nc.sync.dma_start(out=stg[:], in_=stf[:])
                    bnp = bp.tile([128, 2 * NB], f32, tag="bnp")
                    nc.sync.dma_start(out=bnp[:], in_=bn_d[:])
                    mu = bp.tile([128, 6], f32, tag="mu")
                    var = bp.tile([128, 6], f32, tag="var")
                    tmp = bp.tile([128, 6], f32, tag="tmp")
                    inv_n = 1.0 / float(N_REAL)
                    nc.vector.tensor_scalar_mul(mu[:, :NB], stg[:, :NB], inv_n)
                    nc.vector.tensor_scalar_mul(var[:, :NB], stg[:, 6:6 + NB],
                                                inv_n)
                    nc.vector.tensor_tensor(tmp[:, :NB], mu[:, :NB], mu[:, :NB],
                                            OP.mult)
                    nc.vector.tensor_tensor(var[:, :NB], var[:, :NB],
                                            tmp[:, :NB], OP.subtract)
                    nc.vector.tensor_scalar_add(var[:, :NB], var[:, :NB],
                                                BN_EPS)
                    nc.scalar.activation(var[:, :NB], var[:, :NB], AT.Sqrt)
                    nc.vector.reciprocal(var[:, :NB], var[:, :NB])
                    nc.vector.tensor_tensor(sc_out[:, :NB], bnp[:, :NB],
                                            var[:, :NB], OP.mult)
                    nc.vector.tensor_tensor(tmp[:, :NB], mu[:, :NB],
                                            sc_out[:, :NB], OP.mult)
                    nc.vector.tensor_tensor(sh_out[:, :NB], bnp[:, NB:2 * NB],
                                            tmp[:, :NB], OP.subtract)

            def dbg_finish(t128, ncols=None):
                w = min(GPC, ncols or GPC)
                o = pers.tile([1, GPC], f32, tag="dbgy")
                nc.vector.memset(o[:], 0.0)
                nc.vector.tensor_copy(o[:, :w], t128[0:1, 0:w])
                nc.sync.dma_start(out=y_d[:], in_=o[:])

            # ================= Layer 1 =================
            kdims1 = []
            rem = F_IN + 1
            while rem > 0:
                kdims1.append(min(128, rem))
                rem -= kdims1[-1]
            phase1(None, kdims1, W1c_d, CO1, ROW1, h1_locs, False, None, None,
                   "l1")
            if STAGE <= 0:
                dbg_finish(esed[:, 0, :])
                return
            allgather_tbl(h1_locs, h1_full, ROW1, "l1")
            with tc.tile_pool(name="y1", bufs=1) as y1p:
                yT1 = y1p.tile([128, 6, PAD_N], f16, tag="y1", name="y1")
                aggregate(h1_full, ROW1, 256, yT1, 6, "l1")
                if STAGE <= 1:
                    dbg_finish(yT1[:, 0, :])
                    return
                layer_stats(yT1, 6, bn1_d, st_loc[0], st_full[0], sc1, sh1,
                            "l1")
                if STAGE <= 2:
                    dbg_finish(sc1, 6)
                    return

                # ================= Layer 2 =================
                phase1(yT1, [128] * 6, W2c_d, CO23, ROW23, h2_locs, True,
                       sc1, sh1, "l2")
            if STAGE <= 3:
                dbg_finish(esed[:, 0, :])
                return
            allgather_tbl(h2_locs, h2_full, ROW23, "l2")
            with tc.tile_pool(name="y2", bufs=1) as y2p:
                yT2 = y2p.tile([128, 3, PAD_N], f16, tag="y2", name="y2")
                aggregate(h2_full, ROW23, 128, yT2, 3, "l2")
                if STAGE <= 5:
                    dbg_finish(yT2[:, 0, :])
                    return
                layer_stats(yT2, 3, bn2_d, st_loc[1], st_full[1], sc2, sh2,
                            "l2")

                # ================= Layer 3 =================
                phase1(yT2, [128] * 3, W3c_d, CO23, ROW23, h3_locs, True,
                       sc2, sh2, "l3")
            allgather_tbl(h3_locs, h3_full, ROW23, "l3")
            if STAGE <= 7:
                dbg_finish(esed[:, 0, :])
                return
            with tc.tile_pool(name="y3", bufs=1) as y3p:
                yT3 = y3p.tile([128, 3, PAD_N], f16, tag="y3", name="y3")
                aggregate(h3_full, ROW23, 128, yT3, 3, "l3")
                if STAGE <= 8:
                    dbg_finish(yT3[:, 0, :])
                    return
                layer_stats(yT3, 3, bn3_d, st_loc[2], st_full[2], sc3, sh3,
                            "l3")

                # ================= Head =================
                with tc.tile_pool(name="hd", bufs=1) as hp_, \
                     tc.tile_pool(name="hdp", bufs=1, space="PSUM") as pp_:
                    cntb = hp_.tile([128, GPC], f32, tag="cntb")
                    nc.sync.dma_start(out=cntb[:], in_=cntb_d[:])
                    poolT = hp_.tile([128, 3 * GPC], f32, tag="poolT")
                    shc = hp_.tile([128, GPC], f32, tag="shc")
                    for b in range(3):
                        for g_ in range(GPC):
                            nc.vector.tensor_reduce(
                                poolT[:, b * GPC + g_:b * GPC + g_ + 1],
                                yT3[:, b, g_ * GST:(g_ + 1) * GST],
                                AX.X, OP.add)
                        # pool(BN(y)) = pool(y)*sc + cnt_g*sh
                        nc.vector.tensor_scalar(
                            poolT[:, b * GPC:(b + 1) * GPC],
                            poolT[:, b * GPC:(b + 1) * GPC],
                            sc3[:, b:b + 1], None, OP.mult)
                        nc.vector.tensor_scalar(
                            shc[:], cntb[:], sh3[:, b:b + 1], None, OP.mult)
                        nc.vector.tensor_tensor(
                            poolT[:, b * GPC:(b + 1) * GPC],
                            poolT[:, b * GPC:(b + 1) * GPC],
                            shc[:], OP.add)
                    fc1w = hp_.tile([128, 3, 64], f32, tag="fc1w")
                    for b in range(3):
                        nc.sync.dma_start(out=fc1w[:, b, :],
                                          in_=fc1w_d[b * 128:(b + 1) * 128, :])
                    fc2w = hp_.tile([64, 64], f32, tag="fc2w")
                    nc.sync.dma_start(out=fc2w[:], in_=fc2w_d[:])
                    fc3w = hp_.tile([64, 1], f32, tag="fc3w")
                    nc.sync.dma_start(out=fc3w[:], in_=fc3w_d[:])
                    fcb = hp_.tile([64, 3], f32, tag="fcb")
                    nc.sync.dma_start(out=fcb[:], in_=fcb_d[:])
                    bn4 = hp_.tile([64, 2], f32, tag="bn4")
                    nc.sync.dma_start(out=bn4[:], in_=bn4_d[:])

                    p1p = pp_.tile([64, GPC], f32, tag="p1p")
                    for b in range(3):
                        nc.tensor.matmul(p1p[:], fc1w[:, b, :],
                                         poolT[:, b * GPC:(b + 1) * GPC],
                                         start=(b == 0), stop=(b == 2))
                    p1 = hp_.tile([64, GPC], f32, tag="p1")
                    nc.scalar.activation(p1[:], p1p[:], AT.Relu,
                                         bias=fcb[:, 0:1])
                    st4 = hp_.tile([64, 2], f32, tag="st4")
                    scr4 = hp_.tile([64, GPC], f32, tag="scr4")
                    nc.vector.tensor_reduce(st4[:, 0:1], p1[:], AX.X, OP.add)
                    nc.vector.tensor_tensor(scr4[:], p1[:], p1[:], OP.mult)
                    nc.vector.tensor_reduce(st4[:, 1:2], scr4[:], AX.X,
                                            OP.add)
                    nc.sync.dma_start(out=st4_loc[:], in_=st4[:])
                    nc.gpsimd.collective_compute(
                        "AllReduce", OP.add, replica_groups=GROUPS,
                        ins=[st4_loc[:]], outs=[st4_full[:]])
                    st4g = hp_.tile([64, 2], f32, tag="st4g")
                    nc.sync.dma_start(out=st4g[:], in_=st4_full[:])
                    mu4 = hp_.tile([64, 4], f32, tag="mu4")
                    inv_g = 1.0 / float(G)
                    nc.vector.tensor_scalar_mul(mu4[:, 0:2], st4g[:], inv_g)
                    nc.vector.tensor_tensor(mu4[:, 2:3], mu4[:, 0:1],
                                            mu4[:, 0:1], OP.mult)
                    nc.vector.tensor_tensor(mu4[:, 1:2], mu4[:, 1:2],
                                            mu4[:, 2:3], OP.subtract)
                    nc.vector.tensor_scalar_add(mu4[:, 1:2], mu4[:, 1:2],
                                                BN_EPS)
                    nc.scalar.activation(mu4[:, 1:2], mu4[:, 1:2], AT.Sqrt)
                    nc.vector.reciprocal(mu4[:, 1:2], mu4[:, 1:2])
                    nc.vector.tensor_tensor(mu4[:, 2:3], bn4[:, 0:1],
                                            mu4[:, 1:2], OP.mult)
                    nc.vector.tensor_tensor(mu4[:, 3:4], mu4[:, 0:1],
                                            mu4[:, 2:3], OP.mult)
                    nc.vector.tensor_tensor(mu4[:, 3:4], bn4[:, 1:2],
                                            mu4[:, 3:4], OP.subtract)
                    nc.scalar.activation(p1[:], p1[:], AT.Identity,
                                         bias=mu4[:, 3:4], scale=mu4[:, 2:3])
                    p2p = pp_.tile([64, GPC], f32, tag="p2p")
                    nc.tensor.matmul(p2p[:], fc2w[:], p1[:], start=True,
                                     stop=True)
                    p2 = hp_.tile([64, GPC], f32, tag="p2")
                    nc.scalar.activation(p2[:], p2p[:], AT.Relu,
                                         bias=fcb[:, 1:2])
                    p3p = pp_.tile([1, GPC], f32, tag="p3p")
                    nc.tensor.matmul(p3p[:], fc3w[:], p2[:], start=True,
                                     stop=True)
                    p3 = hp_.tile([1, GPC], f32, tag="p3")
                    nc.scalar.activation(p3[:], p3p[:], AT.Relu,
                                         bias=fcb[0:1, 2:3])
                    nc.sync.dma_start(out=y_d[:], in_=p3[:])


def make_in_maps(P):
    in_maps = []
    for c in range(NCORES):
        in_maps.append({
            "x1T": P["x1T"][c], "W1c": P["W1c"], "W2c": P["W2c"],
            "W3c": P["W3c"],
            "src16": P["src16"][c], "o01": P["o01"][c], "o1t": P["o1t"][c],
            "ones1": P["ones1"], "e3": P["e3"], "i128": P["i128"],
            "bn1": P["bn1"], "bn2": P["bn2"], "bn3": P["bn3"], "bn4": P["bn4"],
            "fc1w": P["fc1w"], "fc2w": P["fc2w"], "fc3w": P["fc3w"],
            "fcb": P["fcb"], "cntb": P["cntb"][c],
        })
    return in_maps


def build_program(P):
    from concourse import bass, mybir, tile, bacc, library_config
    nc = bacc.Bacc("TRN2", target_bir_lowering=False, debug=False,
                   num_devices=NCORES)
    _build(nc, P, mybir, tile, bass, library_config)
    nc.compile()
    return nc


def kernel(**inputs):
    from concourse.bass_utils import run_bass_kernel_spmd
    P = _prep(inputs)
    nc = build_program(P)
    res = run_bass_kernel_spmd(nc, make_in_maps(P), list(range(NCORES)))
    out = np.concatenate([res.results[c]["y"][0] for c in range(NCORES)])
    return out[P["gpos"]].astype(np.float32)


# revision 37
# speedup vs baseline: 1.1894x; 1.1894x over previous
"""GAT (3-layer, 3-head) GNN forward on 8 Trainium2 NeuronCores.

Strategy (v2):
- Host partitions the 64 graphs onto 8 cores (8 graphs each); node slots
  padded per graph to a uniform stride so the SPMD program is identical.
- All PE matmuls run in fp16 (1 cycle/row vs 4 for fp32); PSUM stays f32.
- Per layer: phase-1 matmul computes LOCAL node rows [h | es] into a fp16
  DRAM table; the table is AllGathered in S_AG pipelined sub-collectives
  (table ids are sub-block-major so each sub-AllGather's output region is
  contiguous), overlapping the collective with phase-1 compute; BN of the
  previous layer is folded into the weights.
- Aggregation: per 128-dst chunk, dma_gather of h[src] rows; one-hot
  edge->dst matrices are precomputed on host (fp16) and streamed from DRAM
  (both orientations); softmax numerators on ACT with a -8 bias inside exp
  (cancels in the division); weighted one-hots (alpha * onehot) computed
  with wide 4x-mode DVE tensor_scalar ops; scatter-add via fp16 PE matmuls
  accumulating in f32 PSUM; one fused relu*recip finalize op per chunk.
- BN stats: one bn_stats pass over the full layer output at layer end
  (6-tuple -> sum/sumsq algebra), AllReduce, fold into next weights.
- Head: pooling via fixed-stride free-dim reduces, small f32 matmuls, BN4.
"""

import os
import numpy as np

H = 3
NEG_SLOPE = 0.2
BN_EPS = 1e-5
NCORES = 8
EXP_BIAS = -8.0  # subtracted inside exp; cancels in softmax division
S_AG = 2         # sub-AllGathers per layer (pipelined with phase1)

ROW1, ROW23 = 896, 512   # fp16 table row strides (multiples of 128 elems)
CO1, CO23 = 774, 390     # phase-1 matmul widths: h | es(3) | ed(3)


def _ceil_to(x, m):
    return ((x + m - 1) // m) * m


def _prep(inputs):
    """Host-side preprocessing: partitioning, index arrays, one-hot tables,
    combined fp16 weight tables."""
    P = {}
    x = np.asarray(inputs["x"], np.float32)
    ei = np.asarray(inputs["edge_index"])
    batch = np.asarray(inputs["batch"]).astype(np.int64)
    N, F_IN = x.shape
    G = int(batch.max()) + 1
    assert G % NCORES == 0
    GPC = G // NCORES  # graphs per core

    counts = np.bincount(batch, minlength=G)
    gstart = np.concatenate([[0], np.cumsum(counts)[:-1]])
    GSTRIDE = _ceil_to(int(counts.max()), max(16, 128 // GPC))
    PAD_N = GPC * GSTRIDE
    assert PAD_N % 128 == 0
    R = NCORES * PAD_N  # total table rows
    assert R < 32768, f"table rows {R} exceed int16 range"
    CHUNKS = PAD_N // 128
    assert GSTRIDE <= 512, "bn_stats group limit"

    # Balance edge load: graphs sorted by edge count; rank r graph goes to
    # core r%8, slot r//8, so every core's slot s holds a similarly-sized
    # graph and the per-chunk max-over-cores tile count tracks the mean.
    ecnt = np.bincount(batch[np.asarray(ei[1])], minlength=G)
    order = np.argsort(-ecnt, kind="stable")
    gcore = np.empty(G, np.int64)
    gslot = np.empty(G, np.int64)
    gcore[order] = np.arange(G) % NCORES
    gslot[order] = np.arange(G) // NCORES

    n = np.arange(N)
    g = batch
    slot = gcore[g] * PAD_N + gslot[g] * GSTRIDE + (n - gstart[g])
    P.update(N=N, G=G, GPC=GPC, GSTRIDE=GSTRIDE, PAD_N=PAD_N, R=R,
             CHUNKS=CHUNKS, F_IN=F_IN, slot=slot)

    # --- sub-AllGather blocks (tile-aligned); table ids are block-major so
    # each sub-AllGather writes one contiguous region of the full table.
    LNT = CHUNKS
    tsplit = [LNT // S_AG + (1 if i < LNT % S_AG else 0) for i in range(S_AG)]
    rows_s = np.array([t * 128 for t in tsplit], np.int64)
    rowstart = np.concatenate([[0], np.cumsum(rows_s)[:-1]])
    gbase = np.concatenate([[0], np.cumsum(rows_s * NCORES)[:-1]])
    P["ag_tiles"] = tsplit
    P["ag_rowstart"] = rowstart
    P["ag_gbase"] = gbase

    def table_id(sl):
        c, r = sl // PAD_N, sl % PAD_N
        s = np.searchsorted(rowstart, r, side="right") - 1
        return gbase[s] + c * rows_s[s] + (r - rowstart[s])

    # --- edges with self loops, partitioned by dst core, sorted by dst slot
    src = np.concatenate([ei[0], n]).astype(np.int64)
    dst = np.concatenate([ei[1], n]).astype(np.int64)
    sslot = slot[src]
    dslot = slot[dst]
    dcore = dslot // PAD_N

    per_core = []
    for c in range(NCORES):
        m = dcore == c
        ss, dd = table_id(sslot[m]), dslot[m] - c * PAD_N
        order = np.argsort(dd, kind="stable")
        per_core.append((ss[order], (dd[order]) % 128, dd[order] // 128))

    tiles_per_chunk = np.zeros(CHUNKS, np.int64)
    for c in range(NCORES):
        _, _, ck = per_core[c]
        cnt = np.bincount(ck, minlength=CHUNKS)
        tiles_per_chunk = np.maximum(tiles_per_chunk, (cnt + 127) // 128)
    TILES = int(tiles_per_chunk.sum())
    tile_base = np.concatenate([[0], np.cumsum(tiles_per_chunk)[:-1]])
    P["tiles_per_chunk"] = tiles_per_chunk
    P["tile_base"] = tile_base
    P["TILES"] = TILES

    # padded per-core arrays: src table ids (dummy 0), dst offsets (-1)
    src_pad = np.zeros((NCORES, TILES * 128), np.int16)
    dst_pad = np.full((NCORES, TILES * 128), -1, np.int64)
    for c in range(NCORES):
        ss, doff, ck = per_core[c]
        cnt = np.bincount(ck, minlength=CHUNKS)
        off = np.concatenate([[0], np.cumsum(cnt)[:-1]])
        for k in range(CHUNKS):
            nk = int(cnt[k])
            if nk == 0:
                continue
            b = int(tile_base[k]) * 128
            src_pad[c, b:b + nk] = ss[off[k]:off[k] + nk]
            dst_pad[c, b:b + nk] = doff[off[k]:off[k] + nk]

    # wrap src ids for dma_gather: idx i of a chunk-gather at [i%16, i//16],
    # columns grouped per chunk; replicated to 128 partitions.
    IDXC = TILES * 8
    src16 = np.zeros((NCORES, 16, IDXC), np.int16)
    for c in range(NCORES):
        for k in range(CHUNKS):
            t0, nt = int(tile_base[k]), int(tiles_per_chunk[k])
            if nt == 0:
                continue
            seg = src_pad[c, t0 * 128:(t0 + nt) * 128]
            src16[c, :, t0 * 8:(t0 + nt) * 8] = seg.reshape(-1, 16).T
    P["src16"] = np.ascontiguousarray(np.tile(src16, (1, 8, 1)))
    P["IDXC"] = IDXC

    # one-hot edge->dst matrices, both orientations, fp16:
    #   o01[e, t*128+d] = 1 iff edge (t,e) targets chunk-local dst d
    #   o1t[d, t*128+e] = same, transposed per tile
    o01 = np.zeros((NCORES, 128, TILES * 128), np.float16)
    o1t = np.zeros((NCORES, 128, TILES * 128), np.float16)
    idx = np.arange(TILES * 128)
    ee, tt = idx % 128, idx // 128
    for c in range(NCORES):
        d = dst_pad[c]
        v = d >= 0
        o01[c, ee[v], tt[v] * 128 + d[v]] = 1.0
        o1t[c, d[v], tt[v] * 128 + ee[v]] = 1.0
    P["o01"] = o01
    P["o1t"] = o1t

    # --- x^T with ones row, slotted, per-core local slice, fp16
    xT = np.zeros((F_IN + 1, R), np.float16)
    xT[:F_IN, slot] = x.T.astype(np.float16)
    xT[F_IN, slot] = 1.0
    P["x1T"] = np.ascontiguousarray(
        xT.reshape(F_IN + 1, NCORES, PAD_N).transpose(1, 0, 2))

    # --- combined weight tables [K(+1), C*H+6] fp16
    def comb(W, a_s, a_d, b, C, with_bias_row):
        K = W.shape[0]
        rows = K + (1 if with_bias_row else 0)
        Wc = np.zeros((rows, H * C + 6), np.float32)
        Wc[:K, :H * C] = W
        for j in range(H):
            Wc[:K, H * C + j] = W[:, j * C:(j + 1) * C] @ a_s[j]
            Wc[:K, H * C + 3 + j] = W[:, j * C:(j + 1) * C] @ a_d[j]
        if with_bias_row:
            Wc[K, :H * C] = b
        return Wc.astype(np.float16)

    # L1: bias row is matched against the ones row inside x1T (row F_IN),
    # so fold it into the K rows directly at index F_IN.
    W1c = np.zeros((F_IN + 1, H * 256 + 6), np.float32)
    W1c[:F_IN] = comb(np.asarray(inputs["W1"], np.float32),
                      np.asarray(inputs["a1_src"], np.float32),
                      np.asarray(inputs["a1_dst"], np.float32),
                      None, 256, False).astype(np.float32)
    W1c[F_IN, :H * 256] = np.asarray(inputs["b1"], np.float32)
    P["W1c"] = W1c.astype(np.float16)
    P["W2c"] = comb(np.asarray(inputs["W2"], np.float32),
                    np.asarray(inputs["a2_src"], np.float32),
                    np.asarray(inputs["a2_dst"], np.float32),
                    np.asarray(inputs["b2"], np.float32), 128, True)
    P["W3c"] = comb(np.asarray(inputs["W3"], np.float32),
                    np.asarray(inputs["a3_src"], np.float32),
                    np.asarray(inputs["a3_dst"], np.float32),
                    np.asarray(inputs["b3"], np.float32), 128, True)

    def bnpack(gv, bv, nblk):
        t = np.zeros((128, 2 * nblk), np.float32)
        gv = np.asarray(gv, np.float32)
        bv = np.asarray(bv, np.float32)
        for b in range(nblk):
            sg = gv[b * 128:(b + 1) * 128]
            t[:len(sg), b] = sg
            sb = bv[b * 128:(b + 1) * 128]
            t[:len(sb), nblk + b] = sb
        return t

    P["bn1"] = bnpack(inputs["g1"], inputs["be1"], 6)
    P["bn2"] = bnpack(inputs["g2"], inputs["be2"], 3)
    P["bn3"] = bnpack(inputs["g3"], inputs["be3"], 3)
    bn4 = np.zeros((64, 2), np.float32)
    bn4[:, 0] = np.asarray(inputs["g4"], np.float32)
    bn4[:, 1] = np.asarray(inputs["be4"], np.float32)
    P["bn4"] = bn4

    P["fc1w"] = np.asarray(inputs["fc1_w"], np.float32)
    P["fc2w"] = np.asarray(inputs["fc2_w"], np.float32)
    P["fc3w"] = np.asarray(inputs["fc3_w"], np.float32)
    fcb = np.zeros((64, 3), np.float32)
    fcb[:, 0] = np.asarray(inputs["fc1_b"], np.float32)
    fcb[:, 1] = np.asarray(inputs["fc2_b"], np.float32)
    fcb[0, 2] = float(np.asarray(inputs["fc3_b"], np.float32).reshape(-1)[0])
    P["fcb"] = fcb

    g_at = np.empty((NCORES, GPC), np.int64)
    g_at[gcore, gslot] = np.arange(G)
    cntb = np.zeros((NCORES, 128, GPC), np.float32)
    for c in range(NCORES):
        cntb[c, :, :] = counts[g_at[c]][None, :]
    P["cntb"] = cntb
    P["gpos"] = gcore * GPC + gslot  # graph g's position in the raw output
    P["ones1"] = np.ones((1, 128), np.float16)
    e3 = np.zeros((3, 384), np.float16)
    for j in range(3):
        e3[j, j * 128:(j + 1) * 128] = 1.0
    P["e3"] = e3
    P["i128"] = np.eye(128, dtype=np.float16)
    return P


def _build(nc, P, mybir, tile, bass, library_config):
    STAGE = int(os.environ.get("GAT_STAGE", "99"))
    OW_BCAST = int(os.environ.get("GAT_OW_BCAST", "0"))
    dt = mybir.dt
    f32 = dt.float32
    f16 = dt.float16
    AT = mybir.ActivationFunctionType
    OP = mybir.AluOpType
    AX = mybir.AxisListType
    R, PAD_N, CHUNKS, TILES = P["R"], P["PAD_N"], P["CHUNKS"], P["TILES"]
    IDXC, F_IN = P["IDXC"], P["F_IN"]
    GST, GPC = P["GSTRIDE"], P["GPC"]
    LNT = PAD_N // 128
    tiles_per_chunk = P["tiles_per_chunk"]
    tile_base = P["tile_base"]
    N_REAL, G = P["N"], P["G"]
    TMAX = int(tiles_per_chunk.max())
    GROUPS = [list(range(NCORES))]
    ag_tiles = P["ag_tiles"]
    ag_rowstart = P["ag_rowstart"]
    ag_gbase = P["ag_gbase"]

    def block_of_tile(nt):
        r = nt * 128
        s = 0
        while s + 1 < S_AG and r >= ag_rowstart[s + 1]:
            s += 1
        return s

    # ---------------- DRAM tensors ----------------
    def ein(name, shape, dtype=f32):
        return nc.dram_tensor(name, list(shape), dtype, kind="ExternalInput").ap()

    x1T_d = ein("x1T", [F_IN + 1, PAD_N], f16)
    W1c_d = ein("W1c", P["W1c"].shape, f16)
    W2c_d = ein("W2c", P["W2c"].shape, f16)
    W3c_d = ein("W3c", P["W3c"].shape, f16)
    src16_d = ein("src16", [128, IDXC], dt.int16)
    o01_d = ein("o01", [128, TILES * 128], f16)
    o1t_d = ein("o1t", [128, TILES * 128], f16)
    ones1_d = ein("ones1", [1, 128], f16)
    e3_d = ein("e3", [3, 384], f16)
    i128_d = ein("i128", [128, 128], f16)
    bn1_d = ein("bn1", [128, 12])
    bn2_d = ein("bn2", [128, 6])
    bn3_d = ein("bn3", [128, 6])
    bn4_d = ein("bn4", [64, 2])
    fc1w_d = ein("fc1w", [384, 64])
    fc2w_d = ein("fc2w", [64, 64])
    fc3w_d = ein("fc3w", [64, 1])
    fcb_d = ein("fcb", [64, 3])
    cntb_d = ein("cntb", [128, GPC])
    y_d = nc.dram_tensor("y", [1, GPC], f32, kind="ExternalOutput").ap()

    # per-sub-block local tables (fine-grained collective deps) + full tables
    def loc_tbls(lname, row):
        return [nc.dram_tensor(f"{lname}_{s}", [int(ag_tiles[s]) * 128, row],
                               f16).ap() for s in range(S_AG)]

    h1_locs = loc_tbls("h1_loc", ROW1)
    h1_full = nc.dram_tensor("h1_full", [R, ROW1], f16, addr_space="Shared").ap()
    h2_locs = loc_tbls("h2_loc", ROW23)
    h2_full = nc.dram_tensor("h2_full", [R, ROW23], f16, addr_space="Shared").ap()
    h3_locs = loc_tbls("h3_loc", ROW23)
    h3_full = nc.dram_tensor("h3_full", [R, ROW23], f16, addr_space="Shared").ap()
    st_loc = [nc.dram_tensor(f"st{l}_loc", [128, 12], f32).ap() for l in range(3)]
    st_full = [nc.dram_tensor(f"st{l}_full", [128, 12], f32,
                              addr_space="Shared").ap() for l in range(3)]
    st4_loc = nc.dram_tensor("st4_loc", [64, 2], f32).ap()
    st4_full = nc.dram_tensor("st4_full", [64, 2], f32, addr_space="Shared").ap()

    with tile.TileContext(nc) as tc:
        nc.gpsimd.load_library(library_config.mlp)

        with tc.tile_pool(name="pers", bufs=1) as pers:
            ones1 = pers.tile([1, 128], f16, tag="ones1")
            src16 = pers.tile([128, IDXC], dt.int16, tag="src16")
            esed = pers.tile([128, LNT, 6], f16, tag="esed")
            sc1 = pers.tile([128, 6], f32, tag="sc1")
            sh1 = pers.tile([128, 6], f32, tag="sh1")
            sc2 = pers.tile([128, 3], f32, tag="sc2")
            sh2 = pers.tile([128, 3], f32, tag="sh2")
            sc3 = pers.tile([128, 3], f32, tag="sc3")
            sh3 = pers.tile([128, 3], f32, tag="sh3")
            nbias = pers.tile([128, 1], f32, tag="nbias")
            e3 = pers.tile([3, 384], f16, tag="e3")
            i128 = pers.tile([128, 128], f16, tag="i128")
            nc.sync.dma_start(out=ones1[:], in_=ones1_d[:])
            nc.sync.dma_start(out=src16[:], in_=src16_d[:])
            nc.sync.dma_start(out=e3[:], in_=e3_d[:])
            nc.sync.dma_start(out=i128[:], in_=i128_d[:])
            nc.vector.memset(nbias[:], EXP_BIAS)

            # =====================================================
            def phase1(xsrc, kdims, Wc_d, CO, ROW, tbls, with_ones,
                       sc_in, sh_in, tagp):
                """h_tile = lhs @ Wc (+ ones*wbot); writes [h|es] fp16 rows to
                the per-block tbls and es|ed to esed. If sc_in/sh_in given,
                folds the previous layer's BN into the weights first.
                xsrc: None (load x1T from DRAM) or a [128, nK, PAD_N] tile."""
                nK = len(kdims)
                n_mm = (CO + 511) // 512
                with tc.tile_pool(name=f"w{tagp}", bufs=1) as wp, \
                     tc.tile_pool(name=f"p1{tagp}", bufs=3) as sp, \
                     tc.tile_pool(name=f"ps{tagp}", bufs=2, space="PSUM") as pp, \
                     tc.tile_pool(name=f"x{tagp}", bufs=1) as xp:
                    Wts = []
                    r0 = 0
                    for ki, kd in enumerate(kdims):
                        wt = wp.tile([kd, CO], f16, tag=f"w{ki}")
                        nc.sync.dma_start(out=wt[:], in_=Wc_d[r0:r0 + kd, :])
                        Wts.append(wt)
                        r0 += kd
                    wbot = None
                    if with_ones:
                        wbot_raw = wp.tile([1, CO], f16, tag="wbotr")
                        nc.sync.dma_start(out=wbot_raw[:], in_=Wc_d[r0:r0 + 1, :])
                        wbot = wbot_raw
                    if sc_in is not None:
                        # bias fix first (uses raw W): wbot' = wbot + sh @ W
                        shh = wp.tile([128, nK], f16, tag="shh")
                        nc.vector.tensor_copy(shh[:], sh_in[:, :nK])
                        pb = pp.tile([1, CO], f32, tag="pb")
                        for ki in range(nK):
                            nc.tensor.matmul(pb[:], shh[:, ki:ki + 1], Wts[ki][:],
                                             start=(ki == 0), stop=(ki == nK - 1))
                        wbot2 = wp.tile([1, CO], f16, tag="wbot2")
                        nc.vector.tensor_tensor(wbot2[:], wbot_raw[:], pb[:],
                                                OP.add)
                        wbot = wbot2
                        # then scale rows in place: W'[k,:] = sc_k * W[k,:]
                        for ki in range(nK):
                            nc.vector.tensor_scalar(
                                Wts[ki][:], Wts[ki][:], sc_in[:kdims[ki],
                                                              ki:ki + 1],
                                None, OP.mult)

                    if xsrc is None:
                        xg = []
                        r0 = 0
                        for ki, kd in enumerate(kdims):
                            t = xp.tile([kd, PAD_N], f16, tag=f"xg{ki}")
                            nc.sync.dma_start(out=t[:],
                                              in_=x1T_d[r0:r0 + kd, :])
                            xg.append(t)
                            r0 += kd

                    for nt in range(LNT):
                        if xsrc is None:
                            lhs = [xg[ki][:, nt * 128:(nt + 1) * 128]
                                   for ki in range(nK)]
                        else:
                            lhs = [xsrc[:, ki, nt * 128:(nt + 1) * 128]
                                   for ki in range(nK)]
                        hp = pp.tile([128, CO], f32, tag="hp")
                        for ki in range(nK):
                            for mi in range(n_mm):
                                c0, c1 = mi * 512, min(CO, mi * 512 + 512)
                                nc.tensor.matmul(
                                    hp[:, c0:c1], lhs[ki], Wts[ki][:, c0:c1],
                                    start=(ki == 0),
                                    stop=(not with_ones and ki == nK - 1))
                        if with_ones:
                            for mi in range(n_mm):
                                c0, c1 = mi * 512, min(CO, mi * 512 + 512)
                                nc.tensor.matmul(
                                    hp[:, c0:c1], ones1[:], wbot[:, c0:c1],
                                    start=False, stop=True)
                        hs = sp.tile([128, ROW], f16, tag="hs")
                        if nt % 2 == 0:
                            nc.vector.tensor_copy(hs[:, :CO - 3], hp[:, :CO - 3])
                        else:
                            nc.scalar.copy(hs[:, :CO - 3], hp[:, :CO - 3])
                        nc.vector.memset(hs[:, CO - 3:ROW], 0.0)
                        nc.vector.tensor_copy(esed[:, nt, :], hp[:, CO - 6:CO])
                        s = block_of_tile(nt)
                        r0 = nt * 128 - int(ag_rowstart[s])
                        nc.sync.dma_start(
                            out=tbls[s][r0:r0 + 128, :], in_=hs[:])

            def allgather_tbl(tbls, full, ROW, tagp):
                for s in range(S_AG):
                    rows = int(ag_tiles[s]) * 128
                    g0 = int(ag_gbase[s])
                    nc.gpsimd.collective_compute(
                        "AllGather", OP.bypass, replica_groups=GROUPS,
                        ins=[tbls[s][:]],
                        outs=[full[g0:g0 + NCORES * rows, :]])

            # =====================================================
            def aggregate(tbl, ROW, C, yT, NB, tagp):
                """Segment-softmax attention + scatter-add for one layer.
                yT: [128, NB, PAD_N] fp16 output tile (feature-major).
                Two-stage software pipeline: stage A (gather + edge scores +
                denominators) of chunk k+1 is emitted before stage B
                (weighting + scatter-add + finalize) of chunk k so the
                in-order engine queues interleave the two chunks."""
                nbh = C // 128
                with tc.tile_pool(name=f"g{tagp}", bufs=4) as gp, \
                     tc.tile_pool(name=f"o{tagp}", bufs=4) as op_, \
                     tc.tile_pool(name=f"a{tagp}", bufs=3) as ap, \
                     tc.tile_pool(name=f"q{tagp}", bufs=2, space="PSUM") as qa, \
                     tc.tile_pool(name=f"e{tagp}", bufs=2, space="PSUM") as qe, \
                     tc.tile_pool(name=f"r{tagp}", bufs=1, space="PSUM") as qs:

                    def stageA1(k):
                        """Gather + one-hot DMAs + dst-score spread (no DVE,
                        no dependency on the gather)."""
                        T = int(tiles_per_chunk[k])
                        if T == 0:
                            nc.vector.memset(yT[:, :, k * 128:(k + 1) * 128],
                                             0.0)
                            return None
                        t0 = int(tile_base[k])
                        o01c = op_.tile([128, TMAX * 128], f16, tag="o01")
                        o1tc = op_.tile([128, TMAX * 128], f16, tag="o1t")
                        nc.sync.dma_start(
                            out=o01c[:, :T * 128],
                            in_=o01_d[:, t0 * 128:(t0 + T) * 128])
                        nc.sync.dma_start(
                            out=o1tc[:, :T * 128],
                            in_=o1t_d[:, t0 * 128:(t0 + T) * 128])
                        hg = gp.tile([128, TMAX, ROW], f16, tag="hg")
                        nidx = T * 128
                        nc.gpsimd.dma_gather(
                            hg[:, :T, :], tbl[:], src16[:, t0 * 8:(t0 + T) * 8],
                            nidx, nidx, ROW, single_packet=False)
                        edv = esed[:, k, 3:6]
                        edb = qe.tile([128, TMAX, 3], f32, tag="scr")
                        for t in range(T):
                            nc.tensor.matmul(edb[:, t, :],
                                             o1tc[:, t * 128:(t + 1) * 128],
                                             edv, start=(t == 0),
                                             stop=(t == T - 1))
                        return dict(T=T, k=k, o01c=o01c, o1tc=o1tc, hg=hg,
                                    edb=edb)

                    def stageA2(S):
                        """Edge softmax numerators + denominators (consumes
                        the gather; runs one pipeline stage later)."""
                        T, hg, edb, o01c = S["T"], S["hg"], S["edb"], S["o01c"]
                        exc = ap.tile([128, TMAX, 3], f32, tag="exc")
                        nc.vector.tensor_tensor(
                            exc[:, :T, :], hg[:, :T, H * C:H * C + 3],
                            edb[:, :T, :], OP.add)
                        nc.vector.scalar_tensor_tensor(
                            exc[:, :T, :], exc[:, :T, :], NEG_SLOPE,
                            exc[:, :T, :], OP.mult, OP.max)
                        nc.scalar.activation(exc[:, :T, :], exc[:, :T, :],
                                             AT.Exp, bias=nbias[:])
                        # saturating fp16 cast: pad-edge slots read row 0's
                        # src score, whose exp can overflow fp16; inf would
                        # turn the zero one-hot column into NaN (0 * inf)
                        exch = ap.tile([128, TMAX, 3], f16, tag="exch")
                        nc.vector.tensor_scalar_min(exch[:, :T, :],
                                                    exc[:, :T, :], 60000.0)
                        sp_ = qs.tile([3, 128], f32, tag="sp")
                        for t in range(T):
                            nc.tensor.matmul(sp_[:], exch[:, t, :],
                                             o01c[:, t * 128:(t + 1) * 128],
                                             start=(t == 0), stop=(t == T - 1))
                        sr = ap.tile([3, 128], f32, tag="sr")
                        nc.vector.reciprocal(sr[:], sp_[:])
                        # pad dsts have s=0 -> recip inf, and 0*inf = NaN;
                        # clamp (real recips are <= ~3.3e4)
                        nc.vector.tensor_scalar_min(sr[:], sr[:], 60000.0)
                        srh = ap.tile([3, 128], f16, tag="srh")
                        nc.vector.tensor_copy(srh[:], sr[:])
                        S["exch"] = exch
                        S["srh"] = srh

                    def stageB(S):
                        T, o01c, hg, k = S["T"], S["o01c"], S["hg"], S["k"]
                        # alpha-weighted one-hots: one wide broadcast op per
                        # head (in1 free-broadcast of the per-edge alpha)
                        ow = ap.tile([128, TMAX * 3 * 128], f16, tag="ow")
                        o3 = o01c[:, :T * 128].rearrange(
                            "p (t d) -> p t d", t=T)
                        ow4 = ow[:, :T * 384].rearrange(
                            "p (t h d) -> p t h d", h=3, d=128)
                        for h in range(H):
                            nc.vector.tensor_tensor(
                                ow4[:, :, h, :], o3,
                                S["exch"][:, :T, h:h + 1].broadcast_to(
                                    (128, T, 128)), OP.mult)
                        aggp = qa.tile([128, NB, 128], f32, tag="aggp")
                        for t in range(T):
                            for h in range(H):
                                for cb in range(nbh):
                                    fb = h * nbh + cb
                                    # psum groups are per 2KB bank (4 fb
                                    # slices): start/stop only on the bank's
                                    # first/last matmul
                                    st = (t == 0) and (fb % 4 == 0)
                                    sp2 = (t == T - 1) and (
                                        fb % 4 == 3 or fb == NB - 1)
                                    nc.tensor.matmul(
                                        aggp[:, fb, :],
                                        hg[:, t,
                                           h * C + cb * 128:h * C + (cb + 1) * 128],
                                        ow[:, (t * 3 + h) * 128:
                                           (t * 3 + h + 1) * 128],
                                        start=st, stop=sp2)
                        rb = qs.tile([128, 384], f32, tag="rb")
                        for h in range(H):
                            nc.tensor.matmul(rb[:, h * 128:(h + 1) * 128],
                                             e3[:, h * 128:(h + 1) * 128],
                                             S["srh"][:], start=(h == 0),
                                             stop=(h == H - 1))
                        rbs = ap.tile([128, 384], f32, tag="rbs")
                        nc.scalar.copy(rbs[:], rb[:])
                        # fused finalize: yT = relu(aggp) * recip(s);
                        # one 3D op per feature sub-block (verifier caps
                        # tensor_scalar APs at 3 dims)
                        r3 = rbs[:].rearrange("p (h d) -> p h d", h=3)
                        for cb in range(nbh):
                            nc.vector.scalar_tensor_tensor(
                                yT[:, cb:NB:nbh, k * 128:(k + 1) * 128],
                                aggp[:, cb:NB:nbh, :], 0.0, r3,
                                OP.max, OP.mult)

                    states = {}
                    for i in range(CHUNKS + 2):
                        if i < CHUNKS:
                            states[i] = stageA1(i)
                        if 0 <= i - 1 < CHUNKS and states[i - 1] is not None:
                            stageA2(states[i - 1])
                        if 0 <= i - 2 < CHUNKS and states[i - 2] is not None:
                            stageB(states.pop(i - 2))

            # =====================================================
            def layer_stats(yT, NB, bn_d, stl, stf, sc_out, sh_out, tagp):
                """Per-feature sum/sumsq via ACT accum_out -> AllReduce ->
                sc/sh fold coefficients."""
                with tc.tile_pool(name=f"b{tagp}", bufs=1) as bp:
                    scr = bp.tile([128, PAD_N], f16, tag="scr")
                    st2 = bp.tile([128, 12], f32, tag="st2")
                    nc.vector.memset(st2[:], 0.0)
                    for fb in range(NB):
                        nc.scalar.activation(scr[:], yT[:, fb, :], AT.Identity,
                                             accum_out=st2[:, fb:fb + 1])
                        nc.scalar.activation(scr[:], yT[:, fb, :], AT.Square,
                                             accum_out=st2[:, 6 + fb:7 + fb])
                    nc.sync.dma_start(out=stl[:], in_=st2[:])
                    nc.gpsimd.collective_compute(
                        "AllReduce", OP.add, replica_groups=GROUPS,
                        ins=[stl[:]], outs=[stf[:]])
                    stg = bp.tile([128, 12], f32, tag="stg")
                    nc.sync.dma_start(out=stg[:], in_=stf[:])
                    bnp = bp.tile([128, 2 * NB], f32, tag="bnp")
                    nc.sync.dma_start(out=bnp[:], in_=bn_d[:])
                    mu = bp.tile([128, 6], f32, tag="mu")
                    var = bp.tile([128, 6], f32, tag="var")
                    tmp = bp.tile([128, 6], f32, tag="tmp")
                    inv_n = 1.0 / float(N_REAL)
                    nc.vector.tensor_scalar_mul(mu[:, :NB], stg[:, :NB], inv_n)
                    nc.vector.tensor_scalar_mul(var[:, :NB], stg[:, 6:6 + NB],
                                                inv_n)
                    nc.vector.tensor_tensor(tmp[:, :NB], mu[:, :NB], mu[:, :NB],
                                            OP.mult)
                    nc.vector.tensor_tensor(var[:, :NB], var[:, :NB],
                                            tmp[:, :NB], OP.subtract)
                    nc.vector.tensor_scalar_add(var[:, :NB], var[:, :NB],
                                                BN_EPS)
                    nc.scalar.activation(var[:, :NB], var[:, :NB], AT.Sqrt)
                    nc.vector.reciprocal(var[:, :NB], var[:, :NB])
                    nc.vector.tensor_tensor(sc_out[:, :NB], bnp[:, :NB],
                                            var[:, :NB], OP.mult)
                    nc.vector.tensor_tensor(tmp[:, :NB], mu[:, :NB],
                                            sc_out[:, :NB], OP.mult)
                    nc.vector.tensor_tensor(sh_out[:, :NB], bnp[:, NB:2 * NB],
                                            tmp[:, :NB], OP.subtract)

            def dbg_finish(t128, ncols=None):
                w = min(GPC, ncols or GPC)
                o = pers.tile([1, GPC], f32, tag="dbgy")
                nc.vector.memset(o[:], 0.0)
                nc.vector.tensor_copy(o[:, :w], t128[0:1, 0:w])
                nc.sync.dma_start(out=y_d[:], in_=o[:])

            # ================= Layer 1 =================
            kdims1 = []
            rem = F_IN + 1
            while rem > 0:
                kdims1.append(min(128, rem))
                rem -= kdims1[-1]
            phase1(None, kdims1, W1c_d, CO1, ROW1, h1_locs, False, None, None,
                   "l1")
            if STAGE <= 0:
                dbg_finish(esed[:, 0, :])
                return
            allgather_tbl(h1_locs, h1_full, ROW1, "l1")
            with tc.tile_pool(name="y1", bufs=1) as y1p:
                yT1 = y1p.tile([128, 6, PAD_N], f16, tag="y1", name="y1")
                aggregate(h1_full, ROW1, 256, yT1, 6, "l1")
                if STAGE <= 1:
                    dbg_finish(yT1[:, 0, :])
                    return
                layer_stats(yT1, 6, bn1_d, st_loc[0], st_full[0], sc1, sh1,
                            "l1")
                if STAGE <= 2:
                    dbg_finish(sc1, 6)
                    return

                # ================= Layer 2 =================
                phase1(yT1, [128] * 6, W2c_d, CO23, ROW23, h2_locs, True,
                       sc1, sh1, "l2")
            if STAGE <= 3:
                dbg_finish(esed[:, 0, :])
                return
            allgather_tbl(h2_locs, h2_full, ROW23, "l2")
            with tc.tile_pool(name="y2", bufs=1) as y2p:
                yT2 = y2p.tile([128, 3, PAD_N], f16, tag="y2", name="y2")
                aggregate(h2_full, ROW23, 128, yT2, 3, "l2")
                if STAGE <= 5:
                    dbg_finish(yT2[:, 0, :])
                    return
                layer_stats(yT2, 3, bn2_d, st_loc[1], st_full[1], sc2, sh2,
                            "l2")

                # ================= Layer 3 =================
                phase1(yT2, [128] * 3, W3c_d, CO23, ROW23, h3_locs, True,
                       sc2, sh2, "l3")
            allgather_tbl(h3_locs, h3_full, ROW23, "l3")
            if STAGE <= 7:
                dbg_finish(esed[:, 0, :])
                return
            with tc.tile_pool(name="y3", bufs=1) as y3p:
                yT3 = y3p.tile([128, 3, PAD_N], f16, tag="y3", name="y3")
                aggregate(h3_full, ROW23, 128, yT3, 3, "l3")
                if STAGE <= 8:
                    dbg_finish(yT3[:, 0, :])
                    return
                layer_stats(yT3, 3, bn3_d, st_loc[2], st_full[2], sc3, sh3,
                            "l3")

                # ================= Head =================
                with tc.tile_pool(name="hd", bufs=1) as hp_, \
                     tc.tile_pool(name="hdp", bufs=1, space="PSUM") as pp_:
                    cntb = hp_.tile([128, GPC], f32, tag="cntb")
                    nc.sync.dma_start(out=cntb[:], in_=cntb_d[:])
                    poolT = hp_.tile([128, 3 * GPC], f32, tag="poolT")
                    shc = hp_.tile([128, GPC], f32, tag="shc")
                    for b in range(3):
                        for g_ in range(GPC):
                            nc.vector.tensor_reduce(
                                poolT[:, b * GPC + g_:b * GPC + g_ + 1],
                                yT3[:, b, g_ * GST:(g_ + 1) * GST],
                                AX.X, OP.add)
                        # pool(BN(y)) = pool(y)*sc + cnt_g*sh
                        nc.vector.tensor_scalar(
                            poolT[:, b * GPC:(b + 1) * GPC],
                            poolT[:, b * GPC:(b + 1) * GPC],
                            sc3[:, b:b + 1], None, OP.mult)
                        nc.vector.tensor_scalar(
                            shc[:], cntb[:], sh3[:, b:b + 1], None, OP.mult)
                        nc.vector.tensor_tensor(
                            poolT[:, b * GPC:(b + 1) * GPC],
                            poolT[:, b * GPC:(b + 1) * GPC],
                            shc[:], OP.add)
                    fc1w = hp_.tile([128, 3, 64], f32, tag="fc1w")
                    for b in range(3):
                        nc.sync.dma_start(out=fc1w[:, b, :],
                                          in_=fc1w_d[b * 128:(b + 1) * 128, :])
                    fc2w = hp_.tile([64, 64], f32, tag="fc2w")
                    nc.sync.dma_start(out=fc2w[:], in_=fc2w_d[:])
                    fc3w = hp_.tile([64, 1], f32, tag="fc3w")
                    nc.sync.dma_start(out=fc3w[:], in_=fc3w_d[:])
                    fcb = hp_.tile([64, 3], f32, tag="fcb")
                    nc.sync.dma_start(out=fcb[:], in_=fcb_d[:])
                    bn4 = hp_.tile([64, 2], f32, tag="bn4")
                    nc.sync.dma_start(out=bn4[:], in_=bn4_d[:])

                    p1p = pp_.tile([64, GPC], f32, tag="p1p")
                    for b in range(3):
                        nc.tensor.matmul(p1p[:], fc1w[:, b, :],
                                         poolT[:, b * GPC:(b + 1) * GPC],
                                         start=(b == 0), stop=(b == 2))
                    p1 = hp_.tile([64, GPC], f32, tag="p1")
                    nc.scalar.activation(p1[:], p1p[:], AT.Relu,
                                         bias=fcb[:, 0:1])
                    st4 = hp_.tile([64, 2], f32, tag="st4")
                    scr4 = hp_.tile([64, GPC], f32, tag="scr4")
                    nc.vector.tensor_reduce(st4[:, 0:1], p1[:], AX.X, OP.add)
                    nc.vector.tensor_tensor(scr4[:], p1[:], p1[:], OP.mult)
                    nc.vector.tensor_reduce(st4[:, 1:2], scr4[:], AX.X,
                                            OP.add)
                    nc.sync.dma_start(out=st4_loc[:], in_=st4[:])
                    nc.gpsimd.collective_compute(
                        "AllReduce", OP.add, replica_groups=GROUPS,
                        ins=[st4_loc[:]], outs=[st4_full[:]])
                    st4g = hp_.tile([64, 2], f32, tag="st4g")
                    nc.sync.dma_start(out=st4g[:], in_=st4_full[:])
                    mu4 = hp_.tile([64, 4], f32, tag="mu4")
                    inv_g = 1.0 / float(G)
                    nc.vector.tensor_scalar_mul(mu4[:, 0:2], st4g[:], inv_g)
                    nc.vector.tensor_tensor(mu4[:, 2:3], mu4[:, 0:1],
                                            mu4[:, 0:1], OP.mult)
                    nc.vector.tensor_tensor(mu4[:, 1:2], mu4[:, 1:2],
                                            mu4[:, 2:3], OP.subtract)
                    nc.vector.tensor_scalar_add(mu4[:, 1:2], mu4[:, 1:2],
                                                BN_EPS)
                    nc.scalar.activation(mu4[:, 1:2], mu4[:, 1:2], AT.Sqrt)
                    nc.vector.reciprocal(mu4[:, 1:2], mu4[:, 1:2])
                    nc.vector.tensor_tensor(mu4[:, 2:3], bn4[:, 0:1],
                                            mu4[:, 1:2], OP.mult)
                    nc.vector.tensor_tensor(mu4[:, 3:4], mu4[:, 0:1],
                                            mu4[:, 2:3], OP.mult)
                    nc.vector.tensor_tensor(mu4[:, 3:4], bn4[:, 1:2],
                                            mu4[:, 3:4], OP.subtract)
                    nc.scalar.activation(p1[:], p1[:], AT.Identity,
                                         bias=mu4[:, 3:4], scale=mu4[:, 2:3])
                    p2p = pp_.tile([64, GPC], f32, tag="p2p")
                    nc.tensor.matmul(p2p[:], fc2w[:], p1[:], start=True,
                                     stop=True)
                    p2 = hp_.tile([64, GPC], f32, tag="p2")
                    nc.scalar.activation(p2[:], p2p[:], AT.Relu,
                                         bias=fcb[:, 1:2])
                    p3p = pp_.tile([1, GPC], f32, tag="p3p")
                    nc.tensor.matmul(p3p[:], fc3w[:], p2[:], start=True,
                                     stop=True)
                    p3 = hp_.tile([1, GPC], f32, tag="p3")
                    nc.scalar.activation(p3[:], p3p[:], AT.Relu,
                                         bias=fcb[0:1, 2:3])
                    nc.sync.dma_start(out=y_d[:], in_=p3[:])


def make_in_maps(P):
    in_maps = []
    for c in range(NCORES):
        in_maps.append({
            "x1T": P["x1T"][c], "W1c": P["W1c"], "W2c": P["W2c"],
            "W3c": P["W3c"],
            "src16": P["src16"][c], "o01": P["o01"][c], "o1t": P["o1t"][c],
            "ones1": P["ones1"], "e3": P["e3"], "i128": P["i128"],
            "bn1": P["bn1"], "bn2": P["bn2"], "bn3": P["bn3"], "bn4": P["bn4"],
            "fc1w": P["fc1w"], "fc2w": P["fc2w"], "fc3w": P["fc3w"],
            "fcb": P["fcb"], "cntb": P["cntb"][c],
        })
    return in_maps


def build_program(P):
    from concourse import bass, mybir, tile, bacc, library_config
    nc = bacc.Bacc("TRN2", target_bir_lowering=False, debug=False,
                   num_devices=NCORES)
    _build(nc, P, mybir, tile, bass, library_config)
    nc.compile()
    return nc


def kernel(**inputs):
    from concourse.bass_utils import run_bass_kernel_spmd
    P = _prep(inputs)
    nc = build_program(P)
    res = run_bass_kernel_spmd(nc, make_in_maps(P), list(range(NCORES)))
    out = np.concatenate([res.results[c]["y"][0] for c in range(NCORES)])
    return out[P["gpos"]].astype(np.float32)
